# revision 1
# baseline (speedup 1.0000x reference)
"""BertLayer on 8 trn2 NeuronCores — data-parallel over batch (2 per core).

Layout strategy (per core, tokens T=1024 = 2 batches x 512):
  - x is transposed once (PE transpose, bf16) to xT [hidden, tokens].
  - V is produced natural [tokens, hidden] with a ones column per head so the
    attention-context matmul also yields the softmax denominator for free.
  - Q,K are produced transposed per head-pair (qT/kT [hidden, tokens]) and the
    attention for pair t-1 is interleaved with Q/K production of pair t so the
    TensorEngine stays dense (HAM stays at full clock).
  - Scores are computed transposed [keys, queries]; exp is applied by the
    scalar engine on PSUM eviction (scale=1/8 folded in, no max-subtraction:
    inputs are bounded so exp cannot overflow).
  - ctxT = [V|1]^T @ expT accumulates over key tiles; row 64 is sum(exp); the
    reciprocal is broadcast across partitions with a K=1 matmul and applied
    during PSUM eviction.
  - LN1's gamma/beta are folded into W1/b1 on the host, so LN1 emits the
    normalized z directly in bf16 for the FFN transpose; the residual path
    reapplies gamma/beta cheaply off the critical chain.
  - All matmuls run in bf16 (separate LDWEIGHTS path; PSUM accumulate f32);
    weights are converted to bf16 on the host. Residuals/LN stay f32.
"""

import sys

if "/opt/trn_rl_repo" not in sys.path:
    sys.path.insert(0, "/opt/trn_rl_repo")

from contextlib import ExitStack

import ml_dtypes
import numpy as np

import concourse.bass as bass
import concourse.tile as tile
from concourse import bacc, mybir
from concourse.masks import make_identity
from concourse.bass_utils import run_bass_kernel_spmd

F32 = mybir.dt.float32
BF16 = mybir.dt.bfloat16
AF = mybir.ActivationFunctionType
ALU = mybir.AluOpType

# Problem dims (hardcoded: nn_BertLayer, hidden 768, 12 heads, ff 3072)
NB = 16
NCORES = 8
BPC = NB // NCORES
S = 512
T = BPC * S
H = 768
HK = H // 128
NH = 12
HD = 64
FF = 3072
EPS = 1e-12
MT = T // 128
NQ = 3           # ffn chunks
FQ = FF // NQ    # 1024 ff features per chunk
QK = FQ // 128   # 8 k-tiles per chunk
SCALE = 1.0 / float(np.sqrt(HD))


def _bcast_row_ap(vec_ext, n):
    a = vec_ext[:]
    return bass.AP(tensor=a.tensor, offset=a.offset, ap=[[0, 128], [1, n]])


def _col_ap(vec_ext, ntiles):
    a = vec_ext[:]
    return bass.AP(tensor=a.tensor, offset=a.offset, ap=[[1, 128], [128, ntiles]])


def build_nc():
    nc = bacc.Bacc(num_swdge_queues=4)

    x_ext = nc.declare_dram_parameter("hidden_state", [T, H], F32, isOutput=False)
    wq_e = nc.declare_dram_parameter("Wq", [H, H], BF16, isOutput=False)
    bq_e = nc.declare_dram_parameter("bq", [H], F32, isOutput=False)
    wk_e = nc.declare_dram_parameter("Wk", [H, H], BF16, isOutput=False)
    bk_e = nc.declare_dram_parameter("bk", [H], F32, isOutput=False)
    wv_e = nc.declare_dram_parameter("Wv", [H, H], BF16, isOutput=False)
    bv_e = nc.declare_dram_parameter("bv", [H], F32, isOutput=False)
    wo_e = nc.declare_dram_parameter("Wo", [H, H], BF16, isOutput=False)
    bo_e = nc.declare_dram_parameter("bo", [H], F32, isOutput=False)
    l1g_e = nc.declare_dram_parameter("ln1_g", [H], F32, isOutput=False)
    l1b2_e = nc.declare_dram_parameter("ln1b_plus_b2", [H], F32, isOutput=False)
    w1_e = nc.declare_dram_parameter("W1g", [H, FF], BF16, isOutput=False)
    b1_e = nc.declare_dram_parameter("b1f", [FF], F32, isOutput=False)
    w2_e = nc.declare_dram_parameter("W2", [FF, H], BF16, isOutput=False)
    l2g_e = nc.declare_dram_parameter("ln2_g", [H], F32, isOutput=False)
    l2b_e = nc.declare_dram_parameter("ln2_b", [H], F32, isOutput=False)
    out_ext = nc.declare_dram_parameter("out", [T, H], F32, isOutput=True)

    with ExitStack() as top:
        tc = top.enter_context(tile.TileContext(nc))

        const = top.enter_context(tc.tile_pool(name="const", bufs=1))
        small = top.enter_context(tc.tile_pool(name="small", bufs=4))
        ps_full = top.enter_context(tc.tile_pool(name="ps_full", bufs=3, space="PSUM"))
        ps_ctx = top.enter_context(tc.tile_pool(name="ps_ctx", bufs=3, space="PSUM"))
        ps_ffn = top.enter_context(tc.tile_pool(name="ps_ffn", bufs=2, space="PSUM"))
        main = top.enter_context(tc.tile_pool(name="main", bufs=1))
        wpool = top.enter_context(tc.tile_pool(name="wpool", bufs=3))

        ident = const.tile([128, 128], BF16, name="ident")
        make_identity(nc, ident)
        ones_all = const.tile([128, 64], BF16, name="ones_all")
        nc.vector.memset(ones_all, 1.0)
        eps_col = const.tile([128, 1], F32, name="eps_col")
        nc.vector.memset(eps_col, EPS)

        bv_bc = const.tile([128, H], F32, name="bv_bc")
        nc.gpsimd.dma_start(out=bv_bc, in_=_bcast_row_ap(bv_e, H))
        bo_bc = const.tile([128, H], F32, name="bo_bc")
        nc.gpsimd.dma_start(out=bo_bc, in_=_bcast_row_ap(bo_e, H))
        l1g_bc = const.tile([128, H], F32, name="l1g_bc")
        nc.gpsimd.dma_start(out=l1g_bc, in_=_bcast_row_ap(l1g_e, H))
        lb2_bc = const.tile([128, H], F32, name="lb2_bc")
        nc.gpsimd.dma_start(out=lb2_bc, in_=_bcast_row_ap(l1b2_e, H))
        l2g_bc = const.tile([128, H], F32, name="l2g_bc")
        nc.gpsimd.dma_start(out=l2g_bc, in_=_bcast_row_ap(l2g_e, H))
        l2b_bc = const.tile([128, H], F32, name="l2b_bc")
        nc.gpsimd.dma_start(out=l2b_bc, in_=_bcast_row_ap(l2b_e, H))

        bq_cols = const.tile([128, HK], F32, name="bq_cols")
        nc.gpsimd.dma_start(out=bq_cols, in_=_col_ap(bq_e, HK))
        bk_cols = const.tile([128, HK], F32, name="bk_cols")
        nc.gpsimd.dma_start(out=bk_cols, in_=_col_ap(bk_e, HK))
        b1_cols = const.tile([128, FF // 128], F32, name="b1_cols")
        nc.gpsimd.dma_start(out=b1_cols, in_=_col_ap(b1_e, FF // 128))

        # -------- persistent tensors (slots recycled via tags) --------
        xT = main.tile([128, HK, T], BF16, tag="s1", name="xT")
        ctxT = main.tile([128, HK, T], BF16, tag="s2", name="ctxT")
        qT = main.tile([128, HK, T], BF16, tag="s3", bufs=2, name="qT")
        kT = main.tile([128, HK, T], BF16, tag="s4", name="kT")
        vA = main.tile([128, MT, NH, HD + 1], BF16, tag="s5", name="vA")
        nc.vector.memset(vA[:, :, :, HD:HD + 1], 1.0)

        # ---------------- x load + transpose ----------------
        with ExitStack() as ph_ab:
            xload = ph_ab.enter_context(tc.tile_pool(name="xload", bufs=3))
            expp = ph_ab.enter_context(tc.tile_pool(name="expp", bufs=4))
            bcp = ph_ab.enter_context(tc.tile_pool(name="bcp", bufs=3))

            for mt in range(MT):
                xw = xload.tile([128, H], F32, tag="xw", name="xw")
                nc.sync.dma_start(out=xw, in_=x_ext[mt * 128:(mt + 1) * 128, :])
                xwb = xload.tile([128, H], BF16, tag="xwb", name="xwb")
                nc.vector.tensor_copy(out=xwb, in_=xw)
                for c in range(HK):
                    pt = ps_full.tile([128, 128], BF16, tag="ps", name="pt")
                    nc.tensor.transpose(pt, xwb[:, c * 128:(c + 1) * 128], ident)
                    nc.scalar.copy(
                        out=xT[:, c, mt * 128:(mt + 1) * 128], in_=pt)

            # ---------------- V natural (dense PE warmup) ----------------
            wvsb = wpool.tile([128, HK, H], BF16, tag="wsb", name="wvsb")
            for kk in range(HK):
                nc.sync.dma_start(
                    out=wvsb[:, kk, :], in_=wv_e[kk * 128:(kk + 1) * 128, :])
            for mt in range(MT):
                for nt2 in range(2):
                    ps = ps_full.tile([128, 384], F32, tag="ps", name="psv")
                    for kk in range(HK):
                        nc.tensor.matmul(
                            ps,
                            xT[:, kk, mt * 128:(mt + 1) * 128],
                            wvsb[:, kk, nt2 * 384:(nt2 + 1) * 384],
                            start=(kk == 0), stop=(kk == HK - 1),
                        )
                    nc.vector.tensor_add(
                        out=vA[:, mt, nt2 * 6:(nt2 + 1) * 6, 0:HD],
                        in0=ps[:].rearrange("p (h d) -> p h d", d=HD),
                        in1=bv_bc[:, nt2 * 384:(nt2 + 1) * 384].rearrange(
                            "p (h d) -> p h d", d=HD),
                    )

            wqsb = wpool.tile([128, HK, H], BF16, tag="wsb", name="wqsb")
            for kk in range(HK):
                nc.sync.dma_start(
                    out=wqsb[:, kk, :], in_=wq_e[kk * 128:(kk + 1) * 128, :])
            wksb = wpool.tile([128, HK, H], BF16, tag="wsb", name="wksb")
            for kk in range(HK):
                nc.sync.dma_start(
                    out=wksb[:, kk, :], in_=wk_e[kk * 128:(kk + 1) * 128, :])

            def qk_pair(t):
                """Produce qT/kT for head-pair t (hidden cols t*128..)."""
                for (wsb, b_cols, dstT) in ((wqsb, bq_cols, qT),
                                            (wksb, bk_cols, kT)):
                    for nt in range(2):
                        ps = ps_full.tile([128, 512], F32, tag="ps", name="psqk")
                        for kk in range(HK):
                            nc.tensor.matmul(
                                ps,
                                wsb[:, kk, t * 128:(t + 1) * 128],
                                xT[:, kk, nt * 512:(nt + 1) * 512],
                                start=(kk == 0), stop=(kk == HK - 1),
                            )
                        nc.scalar.add(
                            out=dstT[:, t, nt * 512:(nt + 1) * 512],
                            in_=ps[:], add=b_cols[:, t:t + 1],
                        )

            def attend(t):
                """Attention for both batches / both heads of pair t.

                The two heads' softmax denominators are batched into one
                reciprocal at partition rows 0 and 64 (quadrant-aligned), so
                the expensive serial DVE reciprocal runs once per pair."""
                for b in range(BPC):
                    # kt-major, head-minor score emission: the two heads sit
                    # at PE row-groups 0 and 64, so adjacent matmuls run
                    # concurrently in disjoint array halves.
                    expTs = [expp.tile([128, 4, 512], BF16, tag="expT",
                                       name="expT") for _ in range(2)]
                    for kt in range(4):
                        for hh in range(2):
                            poff = hh * 64
                            ps_s = ps_full.tile([128, 512], F32, tag="ps",
                                                name="ps_s")
                            nc.tensor.matmul(
                                ps_s,
                                kT[poff:poff + 64, t,
                                   b * 512 + kt * 128: b * 512 + (kt + 1) * 128],
                                qT[poff:poff + 64, t, b * 512:(b + 1) * 512],
                                start=True, stop=True,
                            )
                            nc.scalar.activation(
                                expTs[hh][:, kt, :], ps_s[:], AF.Exp,
                                scale=float(SCALE))
                    pcs = []
                    for hh in range(2):
                        h = 2 * t + hh
                        ps_c = ps_ctx.tile([HD + 1, 512], F32, tag="ctx",
                                           name="ps_c")
                        for kt in range(4):
                            nc.tensor.matmul(
                                ps_c,
                                vA[:, b * 4 + kt, h, :],
                                expTs[hh][:, kt, :],
                                start=(kt == 0), stop=(kt == 3),
                            )
                        pcs.append(ps_c)
                    sums2 = bcp.tile([128, 512], F32, tag="sums2", bufs=2,
                                     name="sums2")
                    nc.vector.tensor_copy(out=sums2[0:1, :],
                                          in_=pcs[0][HD:HD + 1, :])
                    nc.vector.tensor_copy(out=sums2[64:65, :],
                                          in_=pcs[1][HD:HD + 1, :])
                    rb2 = bcp.tile([128, 512], BF16, tag="rb2", bufs=2,
                                   name="rb2")
                    rec2 = bcp.tile([128, 512], F32, tag="rec2", bufs=2,
                                    name="rec2")
                    nc.vector.reciprocal(rec2[:], sums2[:])
                    nc.vector.tensor_copy(out=rb2[:], in_=rec2[:])
                    for hh in range(2):
                        poff = hh * 64
                        ps_b = ps_ctx.tile([64, 512], F32, tag="ctx", name="ps_b")
                        nc.tensor.matmul(
                            ps_b,
                            ones_all[poff:poff + 1, :],
                            rb2[poff:poff + 1, :],
                            start=True, stop=True)
                        bc_sb = bcp.tile([64, 512], F32, tag="bc_sb",
                                         name="bc_sb")
                        nc.vector.tensor_copy(out=bc_sb[:], in_=ps_b[:])
                        nc.vector.tensor_mul(
                            out=ctxT[poff:poff + 64, t, b * 512:(b + 1) * 512],
                            in0=pcs[hh][0:64, :], in1=bc_sb[:],
                        )

            qk_pair(0)
            for t in range(1, HK):
                attend(t - 1)
                qk_pair(t)
            attend(HK - 1)

        # ------------- Wo + residual + LN1 + h transpose -------------
        # x_nat reuses vA's slot; hT reuses xT's; acc reuses kT's.
        x_nat = main.tile([128, MT, H], BF16, tag="s5", name="x_nat")
        hT = main.tile([128, HK, T], BF16, tag="s1", name="hT")
        acc = main.tile([128, MT, H], F32, tag="s4", name="acc")
        with tc.tile_pool(name="attp", bufs=4) as attp:
            wosb = wpool.tile([128, HK, H], BF16, tag="wsb", name="wosb")
            for kk in range(HK):
                nc.sync.dma_start(
                    out=wosb[:, kk, :], in_=wo_e[kk * 128:(kk + 1) * 128, :])
            for mt in range(MT):
                nc.gpsimd.dma_start(
                    out=x_nat[:, mt, :], in_=x_ext[mt * 128:(mt + 1) * 128, :])
                nc.vector.tensor_add(
                    out=x_nat[:, mt, :], in0=x_nat[:, mt, :], in1=bo_bc[:])
            for mt in range(MT):
                attn = attp.tile([128, H], F32, tag="attn", name="attn")
                for nt2 in range(2):
                    ps = ps_full.tile([128, 384], F32, tag="ps", name="psw")
                    for kk in range(HK):
                        nc.tensor.matmul(
                            ps,
                            ctxT[:, kk, mt * 128:(mt + 1) * 128],
                            wosb[:, kk, nt2 * 384:(nt2 + 1) * 384],
                            start=(kk == 0), stop=(kk == HK - 1),
                        )
                    nc.vector.tensor_add(
                        out=attn[:, nt2 * 384:(nt2 + 1) * 384],
                        in0=ps[:], in1=x_nat[:, mt, nt2 * 384:(nt2 + 1) * 384])
                # LN1 -> z in bf16 directly (gamma/beta folded into W1/b1)
                st = small.tile([128, 3, 6], F32, tag="lnst", bufs=8, name="st")
                for i in range(3):
                    nc.vector.bn_stats(out=st[:, i, :],
                                       in_=attn[:, i * 256:(i + 1) * 256])
                mv = small.tile([128, 2], F32, tag="lnmv", bufs=8, name="mv")
                nc.vector.bn_aggr(out=mv[:], in_=st[:])
                sd = small.tile([128, 1], F32, tag="lnsd", bufs=8, name="sd")
                nc.scalar.activation(sd[:], mv[:, 1:2], AF.Abs_reciprocal_sqrt,
                                     bias=eps_col[:])
                hb = attp.tile([128, H], BF16, tag="hb", name="hb")
                nc.vector.tensor_scalar(
                    out=hb[:], in0=attn[:], scalar1=mv[:, 0:1], scalar2=sd[:],
                    op0=ALU.subtract, op1=ALU.mult,
                )
                for c in range(HK):
                    pt = ps_full.tile([128, 128], BF16, tag="ps", name="pth")
                    nc.tensor.transpose(pt, hb[:, c * 128:(c + 1) * 128], ident)
                    nc.scalar.copy(
                        out=hT[:, c, mt * 128:(mt + 1) * 128], in_=pt)
                # residual path: acc = z*g1 + (ln1_b + b2)   (off critical path)
                nc.vector.tensor_mul(acc[:, mt, :], hb[:], l1g_bc[:])
                nc.vector.tensor_add(acc[:, mt, :], acc[:, mt, :], lb2_bc[:])

        # ---------------- FFN ----------------
        for q in range(NQ):
            w1c = wpool.tile([128, HK, FQ], BF16, tag="wsb", name="w1c")
            for kk in range(HK):
                nc.sync.dma_start(
                    out=w1c[:, kk, :],
                    in_=w1_e[kk * 128:(kk + 1) * 128, q * FQ:(q + 1) * FQ])
            w2c = wpool.tile([128, QK, H], BF16, tag="wsb", name="w2c")
            for kk in range(QK):
                row = (q * QK + kk) * 128
                nc.sync.dma_start(out=w2c[:, kk, :], in_=w2_e[row:row + 128, :])
            gT = main.tile([128, QK, T], BF16, tag="s3", bufs=2,
                           name="gT")  # reuses qT, double-buffered
            for mo in range(QK):
                for nt in range(2):
                    ps = ps_ffn.tile([128, 512], F32, tag="psf", name="psf1")
                    for kk in range(HK):
                        nc.tensor.matmul(
                            ps,
                            w1c[:, kk, mo * 128:(mo + 1) * 128],
                            hT[:, kk, nt * 512:(nt + 1) * 512],
                            start=(kk == 0), stop=(kk == HK - 1),
                        )
                    nc.scalar.activation(
                        gT[:, mo, nt * 512:(nt + 1) * 512], ps[:], AF.Gelu,
                        bias=b1_cols[:, q * QK + mo:q * QK + mo + 1])
            for mt in range(MT):
                for nt2 in range(2):
                    ps = ps_ffn.tile([128, 384], F32, tag="psf", name="psf2")
                    for kk in range(QK):
                        nc.tensor.matmul(
                            ps,
                            gT[:, kk, mt * 128:(mt + 1) * 128],
                            w2c[:, kk, nt2 * 384:(nt2 + 1) * 384],
                            start=(kk == 0), stop=(kk == QK - 1),
                        )
                    nc.vector.tensor_add(
                        out=acc[:, mt, nt2 * 384:(nt2 + 1) * 384],
                        in0=acc[:, mt, nt2 * 384:(nt2 + 1) * 384],
                        in1=ps[:])

        # ---------------- LN2 + store ----------------
        with tc.tile_pool(name="outp", bufs=3) as outp:
            for mt in range(MT):
                src = acc[:, mt, :]
                st = small.tile([128, 3, 6], F32, tag="lnst", bufs=8, name="st2")
                for i in range(3):
                    nc.vector.bn_stats(out=st[:, i, :],
                                       in_=src[:, i * 256:(i + 1) * 256])
                mv = small.tile([128, 2], F32, tag="lnmv", bufs=8, name="mv2")
                nc.vector.bn_aggr(out=mv[:], in_=st[:])
                sd = small.tile([128, 1], F32, tag="lnsd", bufs=8, name="sd2")
                nc.scalar.activation(sd[:], mv[:, 1:2], AF.Abs_reciprocal_sqrt,
                                     bias=eps_col[:])
                ot = outp.tile([128, H], F32, tag="ot", name="ot")
                nc.vector.tensor_scalar(
                    out=ot[:], in0=src, scalar1=mv[:, 0:1], scalar2=sd[:],
                    op0=ALU.subtract, op1=ALU.mult,
                )
                # gamma/beta on the (otherwise idle) gpsimd engine
                nc.gpsimd.tensor_mul(ot[:], ot[:], l2g_bc[:])
                nc.gpsimd.tensor_add(ot[:], ot[:], l2b_bc[:])
                nc.sync.dma_start(
                    out=out_ext[mt * 128:(mt + 1) * 128, :], in_=ot)

    nc.finalize()
    return nc


_NC = None


def _get_nc():
    global _NC
    if _NC is None:
        _NC = build_nc()
    return _NC


def run(inputs, trace=False):
    f32 = lambda n: np.ascontiguousarray(np.asarray(inputs[n], dtype=np.float32))

    def bf16(a):
        return np.ascontiguousarray(a.astype(ml_dtypes.bfloat16))

    hs = f32("hidden_state").reshape(NB, S, H)
    w1 = f32("W1")
    l1g = f32("ln1_g")
    l1b = f32("ln1_b")
    common = {
        "Wq": bf16(f32("Wq")), "bq": f32("bq"),
        "Wk": bf16(f32("Wk")), "bk": f32("bk"),
        "Wv": bf16(f32("Wv")), "bv": f32("bv"),
        "Wo": bf16(f32("Wo")), "bo": f32("bo"),
        "ln1_g": l1g,
        "ln1b_plus_b2": np.ascontiguousarray(l1b + f32("b2")),
        # fold LN1 gamma/beta into the FFN input projection
        "W1g": bf16(l1g[:, None] * w1),
        "b1f": np.ascontiguousarray(f32("b1") + l1b @ w1),
        "W2": bf16(f32("W2")),
        "ln2_g": f32("ln2_g"), "ln2_b": f32("ln2_b"),
    }
    in_maps = []
    for i in range(NCORES):
        m = dict(common)
        m["hidden_state"] = np.ascontiguousarray(
            hs[i * BPC:(i + 1) * BPC].reshape(T, H))
        in_maps.append(m)
    res = run_bass_kernel_spmd(_get_nc(), in_maps, core_ids=list(range(NCORES)),
                               trace=trace)
    out = np.concatenate(
        [res.results[i]["out"].reshape(BPC, S, H) for i in range(NCORES)], axis=0)
    return out, res


def kernel(**inputs):
    return run(inputs)[0]



# revision 14
# speedup vs baseline: 1.2917x; 1.2917x over previous
"""BertLayer on 8 trn2 NeuronCores — data-parallel over batch (2 per core).

Layout strategy (per core, tokens T=1024 = 2 batches x 512):
  - x is transposed once (PE transpose, bf16) to xT [hidden, tokens].
  - V is produced natural [tokens, hidden] with a ones column per head so the
    attention-context matmul also yields the softmax denominator for free.
  - Q,K are produced transposed per head-pair (qT/kT [hidden, tokens]) and the
    attention for pair t-1 is interleaved with Q/K production of pair t so the
    TensorEngine stays dense (HAM stays at full clock).
  - Scores are computed transposed [keys, queries]; exp is applied by the
    scalar engine on PSUM eviction (scale=1/8 folded in, no max-subtraction:
    inputs are bounded so exp cannot overflow).
  - ctxT = [V|1]^T @ expT accumulates over key tiles; row 64 is sum(exp); the
    reciprocal is broadcast across partitions with a K=1 matmul and applied
    during PSUM eviction.
  - LN1's gamma/beta are folded into W1/b1 on the host, so LN1 emits the
    normalized z directly in bf16 for the FFN transpose; the residual path
    reapplies gamma/beta cheaply off the critical chain.
  - All matmuls run in bf16 (separate LDWEIGHTS path; PSUM accumulate f32);
    weights are converted to bf16 on the host. Residuals/LN stay f32.
"""

import sys

if "/opt/trn_rl_repo" not in sys.path:
    sys.path.insert(0, "/opt/trn_rl_repo")

from contextlib import ExitStack

import ml_dtypes
import numpy as np

import concourse.bass as bass
import concourse.tile as tile
from concourse import bacc, mybir
from concourse.masks import make_identity
from concourse.bass_utils import run_bass_kernel_spmd

F32 = mybir.dt.float32
BF16 = mybir.dt.bfloat16
AF = mybir.ActivationFunctionType
ALU = mybir.AluOpType

# Problem dims (hardcoded: nn_BertLayer, hidden 768, 12 heads, ff 3072)
NB = 16
NCORES = 8
BPC = NB // NCORES
S = 512
T = BPC * S
H = 768
HK = H // 128
NH = 12
HD = 64
FF = 3072
EPS = 1e-12
MT = T // 128
NQ = 3           # ffn chunks
FQ = FF // NQ    # 1024 ff features per chunk
QK = FQ // 128   # 8 k-tiles per chunk
SCALE = 1.0 / float(np.sqrt(HD))


def _bcast_row_ap(vec_ext, n):
    a = vec_ext[:]
    return bass.AP(tensor=a.tensor, offset=a.offset, ap=[[0, 128], [1, n]])


def _col_ap(vec_ext, ntiles):
    a = vec_ext[:]
    return bass.AP(tensor=a.tensor, offset=a.offset, ap=[[1, 128], [128, ntiles]])


def build_nc():
    nc = bacc.Bacc(num_swdge_queues=4)

    x_ext = nc.declare_dram_parameter("hidden_state", [T, H], F32, isOutput=False)
    wq_e = nc.declare_dram_parameter("Wq", [H, H], BF16, isOutput=False)
    bq_e = nc.declare_dram_parameter("bq", [H], F32, isOutput=False)
    wk_e = nc.declare_dram_parameter("Wk", [H, H], BF16, isOutput=False)
    bk_e = nc.declare_dram_parameter("bk", [H], F32, isOutput=False)
    wv_e = nc.declare_dram_parameter("Wv", [H, H], BF16, isOutput=False)
    bv_e = nc.declare_dram_parameter("bv", [H], F32, isOutput=False)
    wo_e = nc.declare_dram_parameter("Wo", [H, H], BF16, isOutput=False)
    bo_e = nc.declare_dram_parameter("bo", [H], F32, isOutput=False)
    l1g_e = nc.declare_dram_parameter("ln1_g", [H], F32, isOutput=False)
    l1b2_e = nc.declare_dram_parameter("ln1b_plus_b2", [H], F32, isOutput=False)
    w1_e = nc.declare_dram_parameter("W1g", [H, FF], BF16, isOutput=False)
    b1_e = nc.declare_dram_parameter("b1f", [FF], F32, isOutput=False)
    w2_e = nc.declare_dram_parameter("W2", [FF, H], BF16, isOutput=False)
    l2g_e = nc.declare_dram_parameter("ln2_g", [H], F32, isOutput=False)
    l2b_e = nc.declare_dram_parameter("ln2_b", [H], F32, isOutput=False)
    out_ext = nc.declare_dram_parameter("out", [T, H], F32, isOutput=True)

    with ExitStack() as top:
        tc = top.enter_context(tile.TileContext(nc))

        const = top.enter_context(tc.tile_pool(name="const", bufs=1))
        small = top.enter_context(tc.tile_pool(name="small", bufs=4))
        ps_full = top.enter_context(tc.tile_pool(name="ps_full", bufs=3, space="PSUM"))
        ps_ctx = top.enter_context(tc.tile_pool(name="ps_ctx", bufs=3, space="PSUM"))
        ps_ffn = top.enter_context(tc.tile_pool(name="ps_ffn", bufs=2, space="PSUM"))
        main = top.enter_context(tc.tile_pool(name="main", bufs=1))
        wpool = top.enter_context(tc.tile_pool(name="wpool", bufs=3))

        ident = const.tile([128, 128], BF16, name="ident")
        make_identity(nc, ident)
        ones_all = const.tile([128, 64], BF16, name="ones_all")
        nc.vector.memset(ones_all, 1.0)
        eps_col = const.tile([128, 1], F32, name="eps_col")
        nc.vector.memset(eps_col, EPS)

        bv_bc = const.tile([128, H], F32, name="bv_bc")
        nc.gpsimd.dma_start(out=bv_bc, in_=_bcast_row_ap(bv_e, H))
        bo_bc = const.tile([128, H], F32, name="bo_bc")
        nc.gpsimd.dma_start(out=bo_bc, in_=_bcast_row_ap(bo_e, H))
        l1g_bc = const.tile([128, H], F32, name="l1g_bc")
        nc.gpsimd.dma_start(out=l1g_bc, in_=_bcast_row_ap(l1g_e, H))
        lb2_bc = const.tile([128, H], F32, name="lb2_bc")
        nc.gpsimd.dma_start(out=lb2_bc, in_=_bcast_row_ap(l1b2_e, H))
        l2g_bc = const.tile([128, H], F32, name="l2g_bc")
        nc.gpsimd.dma_start(out=l2g_bc, in_=_bcast_row_ap(l2g_e, H))
        l2b_bc = const.tile([128, H], F32, name="l2b_bc")
        nc.gpsimd.dma_start(out=l2b_bc, in_=_bcast_row_ap(l2b_e, H))

        bq_cols = const.tile([128, HK], F32, name="bq_cols")
        nc.gpsimd.dma_start(out=bq_cols, in_=_col_ap(bq_e, HK))
        bk_cols = const.tile([128, HK], F32, name="bk_cols")
        nc.gpsimd.dma_start(out=bk_cols, in_=_col_ap(bk_e, HK))
        b1_cols = const.tile([128, FF // 128], F32, name="b1_cols")
        nc.gpsimd.dma_start(out=b1_cols, in_=_col_ap(b1_e, FF // 128))

        # -------- persistent tensors (slots recycled via tags) --------
        xT = main.tile([128, HK, T], BF16, tag="s1", name="xT")
        ctxT = main.tile([128, HK, T], BF16, tag="s2", name="ctxT")
        qT = main.tile([128, HK, T], BF16, tag="s3", name="qT")
        kT = main.tile([128, HK, T], BF16, tag="s4", name="kT")
        vA = main.tile([128, MT, NH, HD + 1], BF16, tag="s5", name="vA")
        nc.vector.memset(vA[:, :, :, HD:HD + 1], 1.0)

        # ---------------- x load + transpose ----------------
        with ExitStack() as ph_ab:
            xload = ph_ab.enter_context(tc.tile_pool(name="xload", bufs=2))
            expp = ph_ab.enter_context(tc.tile_pool(name="expp", bufs=3))
            bcp = ph_ab.enter_context(tc.tile_pool(name="bcp", bufs=2))

            for mt in range(MT):
                xw = xload.tile([128, H], F32, tag="xw", name="xw")
                nc.sync.dma_start(out=xw, in_=x_ext[mt * 128:(mt + 1) * 128, :])
                xwb = xload.tile([128, H], BF16, tag="xwb", name="xwb")
                nc.vector.tensor_copy(out=xwb, in_=xw)
                for c in range(HK):
                    pt = ps_full.tile([128, 128], BF16, tag="ps", name="pt")
                    nc.tensor.transpose(pt, xwb[:, c * 128:(c + 1) * 128], ident)
                    nc.scalar.copy(
                        out=xT[:, c, mt * 128:(mt + 1) * 128], in_=pt)

            # ---------------- V natural (dense PE warmup) ----------------
            wvsb = wpool.tile([128, HK, H], BF16, tag="wsb", name="wvsb")
            for kk in range(HK):
                nc.sync.dma_start(
                    out=wvsb[:, kk, :], in_=wv_e[kk * 128:(kk + 1) * 128, :])
            for mt in range(MT):
                for nt2 in range(2):
                    ps = ps_full.tile([128, 384], F32, tag="ps", name="psv")
                    for kk in range(HK):
                        nc.tensor.matmul(
                            ps,
                            xT[:, kk, mt * 128:(mt + 1) * 128],
                            wvsb[:, kk, nt2 * 384:(nt2 + 1) * 384],
                            start=(kk == 0), stop=(kk == HK - 1),
                        )
                    nc.vector.tensor_add(
                        out=vA[:, mt, nt2 * 6:(nt2 + 1) * 6, 0:HD],
                        in0=ps[:].rearrange("p (h d) -> p h d", d=HD),
                        in1=bv_bc[:, nt2 * 384:(nt2 + 1) * 384].rearrange(
                            "p (h d) -> p h d", d=HD),
                    )

            wqsb = wpool.tile([128, HK, H], BF16, tag="wsb", name="wqsb")
            for kk in range(HK):
                nc.sync.dma_start(
                    out=wqsb[:, kk, :], in_=wq_e[kk * 128:(kk + 1) * 128, :])
            wksb = wpool.tile([128, HK, H], BF16, tag="wsb", name="wksb")
            for kk in range(HK):
                nc.sync.dma_start(
                    out=wksb[:, kk, :], in_=wk_e[kk * 128:(kk + 1) * 128, :])
            # W1 streams in during attention (DMA is idle then)
            w1full = wpool.tile([128, HK, FF], BF16, tag="w1", bufs=1,
                                name="w1full")
            for kk in range(HK):
                nc.sync.dma_start(
                    out=w1full[:, kk, :], in_=w1_e[kk * 128:(kk + 1) * 128, :])

            def qk_pair(t):
                """Produce qT/kT for head-pair t (hidden cols t*128..)."""
                for (wsb, b_cols, dstT) in ((wqsb, bq_cols, qT),
                                            (wksb, bk_cols, kT)):
                    for nt in range(2):
                        ps = ps_full.tile([128, 512], F32, tag="ps", name="psqk")
                        for kk in range(HK):
                            nc.tensor.matmul(
                                ps,
                                wsb[:, kk, t * 128:(t + 1) * 128],
                                xT[:, kk, nt * 512:(nt + 1) * 512],
                                start=(kk == 0), stop=(kk == HK - 1),
                            )
                        nc.scalar.add(
                            out=dstT[:, t, nt * 512:(nt + 1) * 512],
                            in_=ps[:], add=b_cols[:, t:t + 1],
                        )

            def attend(t):
                """Attention for both batches / both heads of pair t.

                1/sum(exp) comes from a fast-approx DVE reciprocal applied
                directly to the PSUM denominator row; the broadcast across
                the 64 head partitions rides the (otherwise idle) DMA."""
                for b in range(BPC):
                    # kt-major, head-minor score emission: the two heads sit
                    # at PE row-groups 0 and 64, so adjacent matmuls run
                    # concurrently in disjoint array halves.
                    expTs = [expp.tile([128, 4, 512], BF16, tag="expT",
                                       name="expT") for _ in range(2)]
                    for kt in range(4):
                        for hh in range(2):
                            poff = hh * 64
                            ps_s = ps_full.tile([128, 512], F32, tag="ps",
                                                name="ps_s")
                            nc.tensor.matmul(
                                ps_s,
                                kT[poff:poff + 64, t,
                                   b * 512 + kt * 128: b * 512 + (kt + 1) * 128],
                                qT[poff:poff + 64, t, b * 512:(b + 1) * 512],
                                start=True, stop=True,
                            )
                            nc.scalar.activation(
                                expTs[hh][:, kt, :], ps_s[:], AF.Exp,
                                scale=float(SCALE))
                    pcs = []
                    for hh in range(2):
                        h = 2 * t + hh
                        ps_c = ps_ctx.tile([HD + 1, 512], F32, tag="ctx",
                                           name="ps_c")
                        for kt in range(4):
                            nc.tensor.matmul(
                                ps_c,
                                vA[:, b * 4 + kt, h, :],
                                expTs[hh][:, kt, :],
                                start=(kt == 0), stop=(kt == 3),
                            )
                        pcs.append(ps_c)
                    sums2 = bcp.tile([128, 512], F32, tag="sums2", bufs=2,
                                     name="sums2")
                    nc.vector.tensor_copy(out=sums2[0:1, :],
                                          in_=pcs[0][HD:HD + 1, :])
                    nc.vector.tensor_copy(out=sums2[64:65, :],
                                          in_=pcs[1][HD:HD + 1, :])
                    rb2 = bcp.tile([128, 512], BF16, tag="rb2", bufs=2,
                                   name="rb2")
                    rec2 = bcp.tile([128, 512], F32, tag="rec2", bufs=2,
                                    name="rec2")
                    nc.vector.reciprocal_approx_fast(rec2[:], sums2[:])
                    nc.vector.tensor_copy(out=rb2[:], in_=rec2[:])
                    for hh in range(2):
                        poff = hh * 64
                        ps_b = ps_ctx.tile([64, 512], F32, tag="ctx", name="ps_b")
                        nc.tensor.matmul(
                            ps_b,
                            ones_all[poff:poff + 1, :],
                            rb2[poff:poff + 1, :],
                            start=True, stop=True)
                        bc_sb = bcp.tile([64, 512], F32, tag="bc_sb",
                                         name="bc_sb")
                        nc.vector.tensor_copy(out=bc_sb[:], in_=ps_b[:])
                        nc.vector.tensor_mul(
                            out=ctxT[poff:poff + 64, t, b * 512:(b + 1) * 512],
                            in0=pcs[hh][0:64, :], in1=bc_sb[:],
                        )

            qk_pair(0)
            for t in range(1, HK):
                attend(t - 1)
                qk_pair(t)
            attend(HK - 1)

        # ------------- Wo + residual + LN1 + h transpose -------------
        # x_nat reuses vA's slot; hT reuses xT's; acc reuses kT's.
        # W2 loads here (reusing the attention transients' SBUF range).
        w2pool = top.enter_context(tc.tile_pool(name="w2pool", bufs=1))
        FK = FF // 128
        w2full = w2pool.tile([128, FK, H], BF16, name="w2full")
        for kk in range(FK):
            nc.sync.dma_start(
                out=w2full[:, kk, :], in_=w2_e[kk * 128:(kk + 1) * 128, :])
        x_nat = main.tile([128, MT, H], BF16, tag="s5", name="x_nat")
        hT = main.tile([128, HK, T], BF16, tag="s1", name="hT")
        acc = main.tile([128, MT, H], BF16, tag="s4", name="acc")
        with tc.tile_pool(name="attp", bufs=3) as attp:
            wosb = wpool.tile([128, HK, H], BF16, tag="wsb", name="wosb")
            for kk in range(HK):
                nc.sync.dma_start(
                    out=wosb[:, kk, :], in_=wo_e[kk * 128:(kk + 1) * 128, :])
            for mt in range(MT):
                nc.gpsimd.dma_start(
                    out=x_nat[:, mt, :], in_=x_ext[mt * 128:(mt + 1) * 128, :])
                nc.vector.tensor_add(
                    out=x_nat[:, mt, :], in0=x_nat[:, mt, :], in1=bo_bc[:])
            for mt in range(MT):
                attn = attp.tile([128, H], F32, tag="attn", name="attn")
                for nt2 in range(2):
                    ps = ps_full.tile([128, 384], F32, tag="ps", name="psw")
                    for kk in range(HK):
                        nc.tensor.matmul(
                            ps,
                            ctxT[:, kk, mt * 128:(mt + 1) * 128],
                            wosb[:, kk, nt2 * 384:(nt2 + 1) * 384],
                            start=(kk == 0), stop=(kk == HK - 1),
                        )
                    nc.vector.tensor_add(
                        out=attn[:, nt2 * 384:(nt2 + 1) * 384],
                        in0=ps[:], in1=x_nat[:, mt, nt2 * 384:(nt2 + 1) * 384])
                # LN1 -> z in bf16 directly (gamma/beta folded into W1/b1)
                st = small.tile([128, 3, 6], F32, tag="lnst", bufs=8, name="st")
                for i in range(3):
                    nc.vector.bn_stats(out=st[:, i, :],
                                       in_=attn[:, i * 256:(i + 1) * 256])
                mv = small.tile([128, 2], F32, tag="lnmv", bufs=8, name="mv")
                nc.vector.bn_aggr(out=mv[:], in_=st[:])
                sd = small.tile([128, 1], F32, tag="lnsd", bufs=8, name="sd")
                nc.scalar.activation(sd[:], mv[:, 1:2], AF.Abs_reciprocal_sqrt,
                                     bias=eps_col[:])
                hb = attp.tile([128, H], BF16, tag="hb", name="hb")
                nc.vector.tensor_scalar(
                    out=hb[:], in0=attn[:], scalar1=mv[:, 0:1], scalar2=sd[:],
                    op0=ALU.subtract, op1=ALU.mult,
                )
                for c in range(HK):
                    pt = ps_full.tile([128, 128], BF16, tag="ps", name="pth")
                    nc.tensor.transpose(pt, hb[:, c * 128:(c + 1) * 128], ident)
                    nc.scalar.copy(
                        out=hT[:, c, mt * 128:(mt + 1) * 128], in_=pt)
                # residual path: acc = z*g1 + (ln1_b + b2)   (off critical path)
                nc.vector.tensor_mul(acc[:, mt, :], hb[:], l1g_bc[:])
                nc.vector.tensor_add(acc[:, mt, :], acc[:, mt, :], lb2_bc[:])

        # ---------------- FFN (per batch, full-FF PSUM accumulation) -------
        for b in range(BPC):
            gT = main.tile([128, FK, 512], BF16, tag="s3", name="gT")
            for mo in range(FK):
                ps = ps_ffn.tile([128, 512], F32, tag="psf", name="psf1")
                for kk in range(HK):
                    nc.tensor.matmul(
                        ps,
                        w1full[:, kk, mo * 128:(mo + 1) * 128],
                        hT[:, kk, b * 512:(b + 1) * 512],
                        start=(kk == 0), stop=(kk == HK - 1),
                    )
                nc.scalar.activation(
                    gT[:, mo, :], ps[:], AF.Gelu,
                    bias=b1_cols[:, mo:mo + 1])
            for mtb in range(4):
                mt = b * 4 + mtb
                for nt2 in range(2):
                    ps = ps_ffn.tile([128, 384], F32, tag="psf", name="psf2")
                    for kk in range(FK):
                        nc.tensor.matmul(
                            ps,
                            gT[:, kk, mtb * 128:(mtb + 1) * 128],
                            w2full[:, kk, nt2 * 384:(nt2 + 1) * 384],
                            start=(kk == 0), stop=(kk == FK - 1),
                        )
                    nc.vector.tensor_add(
                        out=acc[:, mt, nt2 * 384:(nt2 + 1) * 384],
                        in0=acc[:, mt, nt2 * 384:(nt2 + 1) * 384],
                        in1=ps[:])

        # ---------------- LN2 + store (rsqrt batched: 1 ACT instr) --------
        with tc.tile_pool(name="outp", bufs=3) as outp:
            mv8 = small.tile([128, MT, 2], F32, tag="lnmv8", name="mv8")
            for mt in range(MT):
                src = acc[:, mt, :]
                st = small.tile([128, 3, 6], F32, tag="lnst", bufs=8, name="st2")
                for i in range(3):
                    nc.vector.bn_stats(out=st[:, i, :],
                                       in_=src[:, i * 256:(i + 1) * 256])
                nc.vector.bn_aggr(out=mv8[:, mt, :], in_=st[:])
            sd8 = small.tile([128, MT], F32, tag="lnsd8", name="sd8")
            nc.scalar.activation(sd8[:], mv8[:, :, 1], AF.Abs_reciprocal_sqrt,
                                 bias=eps_col[:])
            for mt in range(MT):
                ot = outp.tile([128, H], F32, tag="ot", name="ot")
                nc.vector.tensor_scalar(
                    out=ot[:], in0=acc[:, mt, :], scalar1=mv8[:, mt, 0:1],
                    scalar2=sd8[:, mt:mt + 1],
                    op0=ALU.subtract, op1=ALU.mult,
                )
                # gamma/beta on the (otherwise idle) gpsimd engine
                nc.gpsimd.tensor_mul(ot[:], ot[:], l2g_bc[:])
                nc.gpsimd.tensor_add(ot[:], ot[:], l2b_bc[:])
                nc.sync.dma_start(
                    out=out_ext[mt * 128:(mt + 1) * 128, :], in_=ot)

    nc.finalize()
    return nc


_NC = None


def _get_nc():
    global _NC
    if _NC is None:
        _NC = build_nc()
    return _NC


def run(inputs, trace=False):
    f32 = lambda n: np.ascontiguousarray(np.asarray(inputs[n], dtype=np.float32))

    def bf16(a):
        return np.ascontiguousarray(a.astype(ml_dtypes.bfloat16))

    hs = f32("hidden_state").reshape(NB, S, H)
    w1 = f32("W1")
    l1g = f32("ln1_g")
    l1b = f32("ln1_b")
    common = {
        "Wq": bf16(f32("Wq")), "bq": f32("bq"),
        "Wk": bf16(f32("Wk")), "bk": f32("bk"),
        "Wv": bf16(f32("Wv")), "bv": f32("bv"),
        "Wo": bf16(f32("Wo")), "bo": f32("bo"),
        "ln1_g": l1g,
        "ln1b_plus_b2": np.ascontiguousarray(l1b + f32("b2")),
        # fold LN1 gamma/beta into the FFN input projection
        "W1g": bf16(l1g[:, None] * w1),
        "b1f": np.ascontiguousarray(f32("b1") + l1b @ w1),
        "W2": bf16(f32("W2")),
        "ln2_g": f32("ln2_g"), "ln2_b": f32("ln2_b"),
    }
    in_maps = []
    for i in range(NCORES):
        m = dict(common)
        m["hidden_state"] = np.ascontiguousarray(
            hs[i * BPC:(i + 1) * BPC].reshape(T, H))
        in_maps.append(m)
    res = run_bass_kernel_spmd(_get_nc(), in_maps, core_ids=list(range(NCORES)),
                               trace=trace)
    out = np.concatenate(
        [res.results[i]["out"].reshape(BPC, S, H) for i in range(NCORES)], axis=0)
    return out, res


def kernel(**inputs):
    return run(inputs)[0]



# revision 21
# speedup vs baseline: 1.3009x; 1.0071x over previous
"""BertLayer on 8 trn2 NeuronCores — data-parallel over batch (2 per core).

Layout strategy (per core, tokens T=1024 = 2 batches x 512):
  - x is transposed once (PE transpose, bf16) to xT [hidden, tokens].
  - V is produced natural [tokens, hidden] with a ones column per head so the
    attention-context matmul also yields the softmax denominator for free.
  - Q,K are produced transposed per head-pair (qT/kT [hidden, tokens]) and the
    attention for pair t-1 is interleaved with Q/K production of pair t so the
    TensorEngine stays dense (HAM stays at full clock).
  - Scores are computed transposed [keys, queries]; exp is applied by the
    scalar engine on PSUM eviction (scale=1/8 folded in, no max-subtraction:
    inputs are bounded so exp cannot overflow).
  - ctxT = [V|1]^T @ expT accumulates over key tiles; row 64 is sum(exp); the
    reciprocal is broadcast across partitions with a K=1 matmul and applied
    during PSUM eviction.
  - LN1's gamma/beta are folded into W1/b1 on the host, so LN1 emits the
    normalized z directly in bf16 for the FFN transpose; the residual path
    reapplies gamma/beta cheaply off the critical chain.
  - All matmuls run in bf16 (separate LDWEIGHTS path; PSUM accumulate f32);
    weights are converted to bf16 on the host. Residuals/LN stay f32.
"""

import sys

if "/opt/trn_rl_repo" not in sys.path:
    sys.path.insert(0, "/opt/trn_rl_repo")

from contextlib import ExitStack

import ml_dtypes
import numpy as np

import concourse.bass as bass
import concourse.tile as tile
from concourse import bacc, mybir
from concourse.masks import make_identity
from concourse.bass_utils import run_bass_kernel_spmd

F32 = mybir.dt.float32
BF16 = mybir.dt.bfloat16
AF = mybir.ActivationFunctionType
ALU = mybir.AluOpType

# Problem dims (hardcoded: nn_BertLayer, hidden 768, 12 heads, ff 3072)
NB = 16
NCORES = 8
BPC = NB // NCORES
S = 512
T = BPC * S
H = 768
HK = H // 128
NH = 12
HD = 64
FF = 3072
EPS = 1e-12
MT = T // 128
NQ = 3           # ffn chunks
FQ = FF // NQ    # 1024 ff features per chunk
QK = FQ // 128   # 8 k-tiles per chunk
SCALE = 1.0 / float(np.sqrt(HD))


def _bcast_row_ap(vec_ext, n):
    a = vec_ext[:]
    return bass.AP(tensor=a.tensor, offset=a.offset, ap=[[0, 128], [1, n]])


def _col_ap(vec_ext, ntiles):
    a = vec_ext[:]
    return bass.AP(tensor=a.tensor, offset=a.offset, ap=[[1, 128], [128, ntiles]])


def build_nc():
    nc = bacc.Bacc(num_swdge_queues=4)

    x_ext = nc.declare_dram_parameter("hidden_state", [T, H], F32, isOutput=False)
    wq_e = nc.declare_dram_parameter("Wq", [H, H], BF16, isOutput=False)
    bq_e = nc.declare_dram_parameter("bq", [H], F32, isOutput=False)
    wk_e = nc.declare_dram_parameter("Wk", [H, H], BF16, isOutput=False)
    bk_e = nc.declare_dram_parameter("bk", [H], F32, isOutput=False)
    wv_e = nc.declare_dram_parameter("Wv", [H, H], BF16, isOutput=False)
    bv_e = nc.declare_dram_parameter("bv", [H], F32, isOutput=False)
    wo_e = nc.declare_dram_parameter("Wo", [H, H], BF16, isOutput=False)
    bo_e = nc.declare_dram_parameter("bo", [H], F32, isOutput=False)
    l1g_e = nc.declare_dram_parameter("ln1_g", [H], F32, isOutput=False)
    l1b2_e = nc.declare_dram_parameter("ln1b_plus_b2", [H], F32, isOutput=False)
    w1_e = nc.declare_dram_parameter("W1g", [H, FF], BF16, isOutput=False)
    b1_e = nc.declare_dram_parameter("b1f", [FF], F32, isOutput=False)
    w2_e = nc.declare_dram_parameter("W2", [FF, H], BF16, isOutput=False)
    l2g_e = nc.declare_dram_parameter("ln2_g", [H], F32, isOutput=False)
    l2b_e = nc.declare_dram_parameter("ln2_b", [H], F32, isOutput=False)
    out_ext = nc.declare_dram_parameter("out", [T, H], F32, isOutput=True)

    with ExitStack() as top:
        tc = top.enter_context(tile.TileContext(nc))

        const = top.enter_context(tc.tile_pool(name="const", bufs=1))
        small = top.enter_context(tc.tile_pool(name="small", bufs=4))
        ps_full = top.enter_context(tc.tile_pool(name="ps_full", bufs=4, space="PSUM"))
        ps_ctx = top.enter_context(tc.tile_pool(name="ps_ctx", bufs=4, space="PSUM"))
        main = top.enter_context(tc.tile_pool(name="main", bufs=1))
        wpool = top.enter_context(tc.tile_pool(name="wpool", bufs=3))

        ident = const.tile([128, 128], BF16, name="ident")
        make_identity(nc, ident)
        ones_all = const.tile([128, 64], BF16, name="ones_all")
        nc.vector.memset(ones_all, 1.0)
        eps_col = const.tile([128, 1], F32, name="eps_col")
        nc.vector.memset(eps_col, EPS)

        bv_bc = const.tile([128, H], F32, name="bv_bc")
        nc.gpsimd.dma_start(out=bv_bc, in_=_bcast_row_ap(bv_e, H))
        bo_bc = const.tile([128, H], F32, name="bo_bc")
        nc.gpsimd.dma_start(out=bo_bc, in_=_bcast_row_ap(bo_e, H))
        l1g_bc = const.tile([128, H], F32, name="l1g_bc")
        nc.gpsimd.dma_start(out=l1g_bc, in_=_bcast_row_ap(l1g_e, H))
        lb2_bc = const.tile([128, H], F32, name="lb2_bc")
        nc.gpsimd.dma_start(out=lb2_bc, in_=_bcast_row_ap(l1b2_e, H))
        l2g_bc = const.tile([128, H], F32, name="l2g_bc")
        nc.gpsimd.dma_start(out=l2g_bc, in_=_bcast_row_ap(l2g_e, H))
        l2b_bc = const.tile([128, H], F32, name="l2b_bc")
        nc.gpsimd.dma_start(out=l2b_bc, in_=_bcast_row_ap(l2b_e, H))

        bq_cols = const.tile([128, HK], F32, name="bq_cols")
        nc.gpsimd.dma_start(out=bq_cols, in_=_col_ap(bq_e, HK))
        bk_cols = const.tile([128, HK], F32, name="bk_cols")
        nc.gpsimd.dma_start(out=bk_cols, in_=_col_ap(bk_e, HK))
        b1_cols = const.tile([128, FF // 128], F32, name="b1_cols")
        nc.gpsimd.dma_start(out=b1_cols, in_=_col_ap(b1_e, FF // 128))

        # -------- persistent tensors (slots recycled via tags) --------
        xT = main.tile([128, HK, T], BF16, tag="s1", name="xT")
        ctxT = main.tile([128, HK, T], BF16, tag="s2", name="ctxT")
        qT = main.tile([128, HK, T], BF16, tag="s3", name="qT")
        kT = main.tile([128, HK, T], BF16, tag="s4", name="kT")
        vA = main.tile([128, MT, NH, HD + 1], BF16, tag="s5", name="vA")
        x_nat = main.tile([128, MT, H], BF16, tag="s6", name="x_nat")
        nc.vector.memset(vA[:, :, :, HD:HD + 1], 1.0)

        # ---------------- x load + transpose + V (interleaved) ----------
        with ExitStack() as ph_ab:
            xload = ph_ab.enter_context(tc.tile_pool(name="xload", bufs=2))
            expp = ph_ab.enter_context(tc.tile_pool(name="expp", bufs=4))
            bcp = ph_ab.enter_context(tc.tile_pool(name="bcp", bufs=2))

            wvsb = wpool.tile([128, HK, H], BF16, tag="wsb", name="wvsb")
            for kk in range(HK):
                nc.sync.dma_start(
                    out=wvsb[:, kk, :], in_=wv_e[kk * 128:(kk + 1) * 128, :])
            for mt in range(MT):
                xw = xload.tile([128, H], F32, tag="xw", name="xw")
                nc.sync.dma_start(out=xw, in_=x_ext[mt * 128:(mt + 1) * 128, :])
                xwb = xload.tile([128, H], BF16, tag="xwb", name="xwb")
                nc.vector.tensor_copy(out=xwb, in_=xw)
                nc.vector.tensor_add(
                    out=x_nat[:, mt, :], in0=xw[:], in1=bo_bc[:])
                for c in range(HK):
                    pt = ps_full.tile([128, 128], BF16, tag="ps", name="pt")
                    nc.tensor.transpose(pt, xwb[:, c * 128:(c + 1) * 128], ident)
                    nc.scalar.copy(
                        out=xT[:, c, mt * 128:(mt + 1) * 128], in_=pt)
                for nt2 in range(2):
                    ps = ps_full.tile([128, 384], F32, tag="ps", name="psv")
                    for kk in range(HK):
                        nc.tensor.matmul(
                            ps,
                            xT[:, kk, mt * 128:(mt + 1) * 128],
                            wvsb[:, kk, nt2 * 384:(nt2 + 1) * 384],
                            start=(kk == 0), stop=(kk == HK - 1),
                        )
                    nc.vector.tensor_add(
                        out=vA[:, mt, nt2 * 6:(nt2 + 1) * 6, 0:HD],
                        in0=ps[:].rearrange("p (h d) -> p h d", d=HD),
                        in1=bv_bc[:, nt2 * 384:(nt2 + 1) * 384].rearrange(
                            "p (h d) -> p h d", d=HD),
                    )

            wqsb = wpool.tile([128, HK, H], BF16, tag="wsb", name="wqsb")
            for kk in range(HK):
                nc.sync.dma_start(
                    out=wqsb[:, kk, :], in_=wq_e[kk * 128:(kk + 1) * 128, :])
            wksb = wpool.tile([128, HK, H], BF16, tag="wsb", name="wksb")
            for kk in range(HK):
                nc.sync.dma_start(
                    out=wksb[:, kk, :], in_=wk_e[kk * 128:(kk + 1) * 128, :])
            # Wo streams in during attention (DMA is idle then)
            wosb = wpool.tile([128, HK, H], BF16, tag="wsb", name="wosb")
            for kk in range(HK):
                nc.sync.dma_start(
                    out=wosb[:, kk, :], in_=wo_e[kk * 128:(kk + 1) * 128, :])

            def qk_pair(t):
                """Produce qT/kT for head-pair t (hidden cols t*128..)."""
                for (wsb, b_cols, dstT) in ((wqsb, bq_cols, qT),
                                            (wksb, bk_cols, kT)):
                    for nt in range(2):
                        ps = ps_full.tile([128, 512], F32, tag="ps", name="psqk")
                        for kk in range(HK):
                            nc.tensor.matmul(
                                ps,
                                wsb[:, kk, t * 128:(t + 1) * 128],
                                xT[:, kk, nt * 512:(nt + 1) * 512],
                                start=(kk == 0), stop=(kk == HK - 1),
                            )
                        nc.vector.tensor_scalar(
                            out=dstT[:, t, nt * 512:(nt + 1) * 512],
                            in0=ps[:], scalar1=b_cols[:, t:t + 1], scalar2=None,
                            op0=ALU.add,
                        )

            def attend(t):
                """Attention for both batches / both heads of pair t.

                PE order is kept dense: scores for BOTH batches first (exp
                on ACT overlaps the second batch's score matmuls), then ctx
                for both batches, then the reciprocal broadcasts; the DVE
                softmax-apply chain drains under the next pair's qk work."""
                expT_all = []
                for b in range(BPC):
                    # kt-major, head-minor score emission: the two heads sit
                    # at PE row-groups 0 and 64, so adjacent matmuls run
                    # concurrently in disjoint array halves.
                    expTs = [expp.tile([128, 4, 512], BF16, tag="expT",
                                       name="expT") for _ in range(2)]
                    for kt in range(4):
                        for hh in range(2):
                            poff = hh * 64
                            ps_s = ps_full.tile([128, 512], F32, tag="ps",
                                                name="ps_s")
                            nc.tensor.matmul(
                                ps_s,
                                kT[poff:poff + 64, t,
                                   b * 512 + kt * 128: b * 512 + (kt + 1) * 128],
                                qT[poff:poff + 64, t, b * 512:(b + 1) * 512],
                                start=True, stop=True,
                            )
                            nc.scalar.activation(
                                expTs[hh][:, kt, :], ps_s[:], AF.Exp,
                                scale=float(SCALE))
                    expT_all.append(expTs)
                pcs_all = []
                for b in range(BPC):
                    pcs = []
                    for hh in range(2):
                        h = 2 * t + hh
                        ps_c = ps_ctx.tile([HD + 1, 512], F32, tag="ctx",
                                           name="ps_c")
                        for kt in range(4):
                            nc.tensor.matmul(
                                ps_c,
                                vA[:, b * 4 + kt, h, :],
                                expT_all[b][hh][:, kt, :],
                                start=(kt == 0), stop=(kt == 3),
                            )
                        pcs.append(ps_c)
                    pcs_all.append(pcs)
                rb2_all = []
                for b in range(BPC):
                    pcs = pcs_all[b]
                    sums2 = bcp.tile([128, 512], F32, tag="sums2", bufs=2,
                                     name="sums2")
                    nc.vector.tensor_copy(out=sums2[0:1, :],
                                          in_=pcs[0][HD:HD + 1, :])
                    nc.vector.tensor_copy(out=sums2[64:65, :],
                                          in_=pcs[1][HD:HD + 1, :])
                    rb2 = bcp.tile([128, 512], BF16, tag="rb2", bufs=2,
                                   name="rb2")
                    rec2 = bcp.tile([128, 512], F32, tag="rec2", bufs=2,
                                    name="rec2")
                    nc.vector.reciprocal_approx_fast(rec2[:], sums2[:])
                    nc.vector.tensor_copy(out=rb2[:], in_=rec2[:])
                    rb2_all.append(rb2)
                for b in range(BPC):
                    for hh in range(2):
                        poff = hh * 64
                        ps_b = ps_full.tile([64, 512], F32, tag="ps",
                                            name="ps_b")
                        nc.tensor.matmul(
                            ps_b,
                            ones_all[poff:poff + 1, :],
                            rb2_all[b][poff:poff + 1, :],
                            start=True, stop=True)
                        bc_sb = bcp.tile([64, 512], F32, tag="bc_sb",
                                         name="bc_sb")
                        nc.vector.tensor_copy(out=bc_sb[:], in_=ps_b[:])
                        nc.vector.tensor_mul(
                            out=ctxT[poff:poff + 64, t, b * 512:(b + 1) * 512],
                            in0=pcs_all[b][hh][0:64, :], in1=bc_sb[:],
                        )

            qk_pair(0)
            for t in range(1, HK):
                attend(t - 1)
                qk_pair(t)
            attend(HK - 1)

        # ------------- Wo + residual + LN1 + h transpose -------------
        # x_nat reuses vA's slot; hT reuses xT's; acc reuses kT's.
        # W2 loads here (reusing the attention transients' SBUF range).
        w2pool = top.enter_context(tc.tile_pool(name="w2pool", bufs=1))
        FK = FF // 128
        w2full = w2pool.tile([128, FK, H], BF16, name="w2full")
        for kk in range(FK):
            nc.sync.dma_start(
                out=w2full[:, kk, :], in_=w2_e[kk * 128:(kk + 1) * 128, :])
        hT = main.tile([128, HK, T], BF16, tag="s1", name="hT")
        acc = main.tile([128, MT, H], BF16, tag="s4", name="acc")
        with tc.tile_pool(name="attp", bufs=3) as attp:

            def h_transpose(hb, mt):
                for c in range(HK):
                    pt = ps_full.tile([128, 128], BF16, tag="ps", name="pth")
                    nc.tensor.transpose(pt, hb[:, c * 128:(c + 1) * 128], ident)
                    nc.scalar.copy(
                        out=hT[:, c, mt * 128:(mt + 1) * 128], in_=pt)

            prev = None
            for mt in range(MT):
                attn = attp.tile([128, H], F32, tag="attn", bufs=2, name="attn")
                for nt2 in range(2):
                    ps = ps_full.tile([128, 384], F32, tag="ps", name="psw")
                    for kk in range(HK):
                        nc.tensor.matmul(
                            ps,
                            ctxT[:, kk, mt * 128:(mt + 1) * 128],
                            wosb[:, kk, nt2 * 384:(nt2 + 1) * 384],
                            start=(kk == 0), stop=(kk == HK - 1),
                        )
                    nc.vector.tensor_add(
                        out=attn[:, nt2 * 384:(nt2 + 1) * 384],
                        in0=ps[:], in1=x_nat[:, mt, nt2 * 384:(nt2 + 1) * 384])
                # transposes of the previous tile fill PE while this tile's
                # LN1 chain (DVE stats -> ACT rsqrt -> DVE apply) drains
                if prev is not None:
                    h_transpose(*prev)
                # LN1 -> z in bf16 directly (gamma/beta folded into W1/b1)
                st = small.tile([128, 3, 6], F32, tag="lnst", bufs=8, name="st")
                for i in range(3):
                    nc.vector.bn_stats(out=st[:, i, :],
                                       in_=attn[:, i * 256:(i + 1) * 256])
                mv = small.tile([128, 2], F32, tag="lnmv", bufs=8, name="mv")
                nc.vector.bn_aggr(out=mv[:], in_=st[:])
                sd = small.tile([128, 1], F32, tag="lnsd", bufs=8, name="sd")
                nc.scalar.activation(sd[:], mv[:, 1:2], AF.Abs_reciprocal_sqrt,
                                     bias=eps_col[:])
                hb = attp.tile([128, H], BF16, tag="hb", name="hb")
                nc.vector.tensor_scalar(
                    out=hb[:], in0=attn[:], scalar1=mv[:, 0:1], scalar2=sd[:],
                    op0=ALU.subtract, op1=ALU.mult,
                )
                # residual path: acc = z*g1 + (ln1_b + b2) on the idle gpsimd
                nc.gpsimd.tensor_mul(acc[:, mt, :], hb[:], l1g_bc[:])
                nc.gpsimd.tensor_add(acc[:, mt, :], acc[:, mt, :], lb2_bc[:])
                prev = (hb, mt)
            h_transpose(*prev)

        # ---------------- FFN (per batch, full-FF PSUM accumulation) -------
        for b in range(BPC):
            gT = main.tile([128, FK, 512], BF16, tag="s3", name="gT")
            for q in range(NQ):
                w1c = wpool.tile([128, HK, FQ], BF16, tag="w1", bufs=2,
                                 name="w1c")
                for kk in range(HK):
                    nc.sync.dma_start(
                        out=w1c[:, kk, :],
                        in_=w1_e[kk * 128:(kk + 1) * 128, q * FQ:(q + 1) * FQ])
                for moq in range(QK):
                    mo = q * QK + moq
                    ps = ps_full.tile([128, 512], F32, tag="ps", name="psf1")
                    for kk in range(HK):
                        nc.tensor.matmul(
                            ps,
                            w1c[:, kk, moq * 128:(moq + 1) * 128],
                            hT[:, kk, b * 512:(b + 1) * 512],
                            start=(kk == 0), stop=(kk == HK - 1),
                        )
                    nc.scalar.activation(
                        gT[:, mo, :], ps[:], AF.Gelu,
                        bias=b1_cols[:, mo:mo + 1])
            for mtb in range(4):
                mt = b * 4 + mtb
                for nt2 in range(2):
                    ps = ps_full.tile([128, 384], F32, tag="ps", name="psf2")
                    for kk in range(FK):
                        nc.tensor.matmul(
                            ps,
                            gT[:, kk, mtb * 128:(mtb + 1) * 128],
                            w2full[:, kk, nt2 * 384:(nt2 + 1) * 384],
                            start=(kk == 0), stop=(kk == FK - 1),
                        )
                    nc.vector.tensor_add(
                        out=acc[:, mt, nt2 * 384:(nt2 + 1) * 384],
                        in0=acc[:, mt, nt2 * 384:(nt2 + 1) * 384],
                        in1=ps[:])

        # ---------------- LN2 + store (rsqrt batched: 1 ACT instr) --------
        with tc.tile_pool(name="outp", bufs=3) as outp:
            mv8 = small.tile([128, MT, 2], F32, tag="lnmv8", name="mv8")
            for mt in range(MT):
                src = acc[:, mt, :]
                st = small.tile([128, 3, 6], F32, tag="lnst", bufs=8, name="st2")
                for i in range(3):
                    nc.vector.bn_stats(out=st[:, i, :],
                                       in_=src[:, i * 256:(i + 1) * 256])
                nc.vector.bn_aggr(out=mv8[:, mt, :], in_=st[:])
            sd8 = small.tile([128, MT], F32, tag="lnsd8", name="sd8")
            nc.scalar.activation(sd8[:], mv8[:, :, 1], AF.Abs_reciprocal_sqrt,
                                 bias=eps_col[:])
            for mt in range(MT):
                ot = outp.tile([128, H], F32, tag="ot", name="ot")
                nc.vector.tensor_scalar(
                    out=ot[:], in0=acc[:, mt, :], scalar1=mv8[:, mt, 0:1],
                    scalar2=sd8[:, mt:mt + 1],
                    op0=ALU.subtract, op1=ALU.mult,
                )
                # gamma/beta on the (otherwise idle) gpsimd engine
                nc.gpsimd.tensor_mul(ot[:], ot[:], l2g_bc[:])
                nc.gpsimd.tensor_add(ot[:], ot[:], l2b_bc[:])
                nc.sync.dma_start(
                    out=out_ext[mt * 128:(mt + 1) * 128, :], in_=ot)

    nc.finalize()
    return nc


_NC = None


def _get_nc():
    global _NC
    if _NC is None:
        _NC = build_nc()
    return _NC


def run(inputs, trace=False):
    f32 = lambda n: np.ascontiguousarray(np.asarray(inputs[n], dtype=np.float32))

    def bf16(a):
        return np.ascontiguousarray(a.astype(ml_dtypes.bfloat16))

    hs = f32("hidden_state").reshape(NB, S, H)
    w1 = f32("W1")
    l1g = f32("ln1_g")
    l1b = f32("ln1_b")
    common = {
        "Wq": bf16(f32("Wq")), "bq": f32("bq"),
        "Wk": bf16(f32("Wk")), "bk": f32("bk"),
        "Wv": bf16(f32("Wv")), "bv": f32("bv"),
        "Wo": bf16(f32("Wo")), "bo": f32("bo"),
        "ln1_g": l1g,
        "ln1b_plus_b2": np.ascontiguousarray(l1b + f32("b2")),
        # fold LN1 gamma/beta into the FFN input projection
        "W1g": bf16(l1g[:, None] * w1),
        "b1f": np.ascontiguousarray(f32("b1") + l1b @ w1),
        "W2": bf16(f32("W2")),
        "ln2_g": f32("ln2_g"), "ln2_b": f32("ln2_b"),
    }
    in_maps = []
    for i in range(NCORES):
        m = dict(common)
        m["hidden_state"] = np.ascontiguousarray(
            hs[i * BPC:(i + 1) * BPC].reshape(T, H))
        in_maps.append(m)
    res = run_bass_kernel_spmd(_get_nc(), in_maps, core_ids=list(range(NCORES)),
                               trace=trace)
    out = np.concatenate(
        [res.results[i]["out"].reshape(BPC, S, H) for i in range(NCORES)], axis=0)
    return out, res


def kernel(**inputs):
    return run(inputs)[0]

